# revision 11
# baseline (speedup 1.0000x reference)
"""Trainium2 Bass kernel for nn_BQ_CorrBlock (ball-query + corr gather + conv/GN/PReLU/max/conv).

Sharding: 8 cores = (batch b in 0..3) x (query-half h in 0..1). Each core handles
2048 queries x 4096 points. GroupNorm statistics span a full batch item, so the
two cores of a pair AllReduce their per-group partial sums (16 floats).

Algorithm per core (16 query tiles of 128):
  - PE computes g[q,p] = r^2 - |c|^2 + 2 c.x - |x|^2  (positive iff inside ball)
    via a K=5 matmul against host-prepared augmented coordinates.
  - score = (g > 0) * (N - p)  in one fused scalar_tensor_tensor; DVE max8 gives
    the 8 smallest valid point indices per query (PointNet++ ball query order).
  - Indices round-trip through DRAM to build the 16-partition-wrapped, group-
    replicated index tensor that gpsimd indirect_copy expects; two gathers pull
    f2 columns [128d x 1024] and xyz2 components for all (q,s) pairs.
  - cur_corr = ones^T (f1_scaled * f2cols) via elementwise mult + K=128 matmul.
  - 1x1 conv (+ bias - W123.c folded in) as one K=8 matmul -> x[64, (q,s)].
  - GroupNorm stats accumulated via activation accum_out; pair AllReduce;
    normalize+PReLU in one activation; max over s; final 64x64 matmul + bias.
"""

import os
import sys

import numpy as np

for _p in ("/opt/trn_rl_repo", "/root/.axon_site/_ro/trn_rl_repo"):
    if os.path.isdir(_p) and _p not in sys.path:
        sys.path.append(_p)

RADIUS = 1.0
NS = 8
GROUPS = 8
EPS = 1e-5
B, N, DIM, C = 4, 4096, 128, 64
HALF = N // 2          # queries per core
NT = HALF // 128       # query tiles per core
CNT = (C // GROUPS) * N * NS  # elements per (batch, group) for GN stats


def _build(prelu_alpha: float):
    import concourse.bacc as bacc
    import concourse.mybir as mybir
    import concourse.tile as tile

    f32 = mybir.dt.float32
    u16 = mybir.dt.uint16
    Alu = mybir.AluOpType
    Act = mybir.ActivationFunctionType

    nc = bacc.Bacc(None)

    d_gq = nc.dram_tensor("gq", [5, HALF], f32, kind="ExternalInput")
    d_gp = nc.dram_tensor("gp", [5, N], f32, kind="ExternalInput")
    d_f1 = nc.dram_tensor("f1s", [128, HALF], f32, kind="ExternalInput")
    d_f2 = nc.dram_tensor("f2", [128, N], f32, kind="ExternalInput")
    d_xyz = nc.dram_tensor("xyzd", [4, N], f32, kind="ExternalInput")
    d_wcc = nc.dram_tensor("wcc", [1, C], f32, kind="ExternalInput")
    d_wcx = nc.dram_tensor("wcx", [3, C], f32, kind="ExternalInput")
    d_wcatb = nc.dram_tensor("wcatb", [5, C], f32, kind="ExternalInput")
    d_wout = nc.dram_tensor("wout", [C, C], f32, kind="ExternalInput")
    d_gnab = nc.dram_tensor("gnab", [C, 2], f32, kind="ExternalInput")
    d_outb = nc.dram_tensor("outb", [C, 1], f32, kind="ExternalInput")
    d_gsel = nc.dram_tensor("gsel", [C, GROUPS], f32, kind="ExternalInput")
    d_gselT = nc.dram_tensor("gselT", [GROUPS, C], f32, kind="ExternalInput")
    d_ones = nc.dram_tensor("ones1", [128, 1], f32, kind="ExternalInput")
    d_ident = nc.dram_tensor("ident", [128, 128], f32, kind="ExternalInput")
    d_out = nc.dram_tensor("out", [C, HALF], f32, kind="ExternalOutput")

    with tile.TileContext(nc) as tc:
        with (
            tc.tile_pool(name="cst", bufs=1) as cst,
            tc.tile_pool(name="score", bufs=1) as score_pool,
            tc.tile_pool(name="small", bufs=2) as small,
            tc.tile_pool(name="gath", bufs=3) as gath,
            tc.tile_pool(name="xtp", bufs=1) as xt_pool,
            tc.tile_pool(name="ph2", bufs=2) as ph2,
            tc.tile_pool(name="gps", bufs=3, space="PSUM") as gpsum_pool,
            tc.tile_pool(name="ccp", bufs=1, space="PSUM") as cc_pool,
            tc.tile_pool(name="xtps", bufs=1, space="PSUM") as xtp_pool,
            tc.tile_pool(name="ops", bufs=1, space="PSUM") as op_pool,
            tc.tile_pool(name="drm", bufs=2, space="DRAM") as drm,
        ):
            # ---- resident constants ----
            gqs = cst.tile([5, HALF], f32)
            gps = cst.tile([5, N], f32)
            f1s = cst.tile([128, HALF], f32)
            f2s = cst.tile([128, N], f32)
            xyzs = cst.tile([128, N], f32)
            wccs = cst.tile([1, C], f32)
            wcxs = cst.tile([3, C], f32)
            wcatbs = cst.tile([5, C], f32)
            wouts = cst.tile([C, C], f32)
            gnabs = cst.tile([C, 2], f32)
            outbs = cst.tile([C, 1], f32)
            gsels = cst.tile([C, GROUPS], f32)
            gselTs = cst.tile([GROUPS, C], f32)
            ones1 = cst.tile([128, 1], f32)
            ident = cst.tile([128, 128], f32)
            nmj = cst.tile([128, N], mybir.dt.int16)

            for t, d in [(gqs, d_gq), (gps, d_gp), (f1s, d_f1),
                         (f2s, d_f2), (wccs, d_wcc), (wcxs, d_wcx), (wcatbs, d_wcatb), (wouts, d_wout),
                         (gnabs, d_gnab), (outbs, d_outb), (gsels, d_gsel),
                         (gselTs, d_gselT), (ones1, d_ones), (ident, d_ident)]:
                nc.sync.dma_start(t[:], d[:])
            nc.gpsimd.memset(xyzs[:], 0)
            nc.sync.dma_start(xyzs[0:4, :], d_xyz[:])
            # nmj[p, j] = N - j (same on every partition); exact in fp32
            nc.gpsimd.iota(nmj[:], pattern=[[-1, N]], base=N, channel_multiplier=0,
                           allow_small_or_imprecise_dtypes=True)

            s1c = cst.tile([C, NT], f32)   # per-tile sums of x
            s2c = cst.tile([C, NT], f32)   # per-tile sums of x^2
            xtall = cst.tile([C, NT * 1024], f32)  # conv outputs, resident

            # ---------------- phase 1 (3-stage software pipeline) ----------------
            # Stage A(i): all-pairs metric + score.  Stage B(i): top-8 + index
            # math + wrapped-index build (PE transpose, DMA bounce).  Stage
            # C(i): gathers + cur_corr + conv + stats.  Emitting A(i), B(i-1),
            # C(i-2) per iteration gives each cross-engine hop a full stage of
            # slack, so the DMA/gather latency hides behind the next tile's
            # score work.
            st_score = [None] * NT
            st_idxf = [None] * NT
            st_wrep = [None] * NT

            def stage_a(i):
                q0 = i * 128
                score = score_pool.tile([128, N], f32, tag="score")
                st_score[i] = score
                for s in range(8):
                    gp_ps = gpsum_pool.tile([128, 512], f32, tag="gps")
                    nc.tensor.matmul(gp_ps[:], gqs[:, q0:q0 + 128],
                                     gps[:, s * 512:(s + 1) * 512])
                    nc.vector.scalar_tensor_tensor(
                        score[:, s * 512:(s + 1) * 512], gp_ps[:], 0.0,
                        nmj[:, s * 512:(s + 1) * 512],
                        op0=Alu.is_gt, op1=Alu.mult)

            def stage_b(i):
                score = st_score[i]
                s8 = small.tile([128, NS], f32, tag="s8")
                nc.vector.max(out=s8[:], in_=score[:])
                valid = small.tile([128, NS], f32, tag="valid")
                nc.vector.tensor_scalar(valid[:], s8[:], 0.5, scalar2=None,
                                        op0=Alu.is_gt)
                idxf = small.tile([128, NS], f32, tag="idxf")
                st_idxf[i] = idxf
                nc.vector.tensor_scalar(idxf[:], s8[:], -1.0, scalar2=float(N),
                                        op0=Alu.mult, op1=Alu.add)
                nc.vector.tensor_tensor(out=idxf[:], in0=idxf[:], in1=valid[:],
                                        op=Alu.mult)
                pad = small.tile([128, 1], f32, tag="pad")
                nc.vector.tensor_copy(pad[:], idxf[:, 0:1])
                # idx = valid ? idxf : pad  ==  pad + valid*(idxf - pad)
                nc.vector.tensor_scalar(idxf[:], idxf[:], pad[:], scalar2=None,
                                        op0=Alu.subtract)
                nc.vector.tensor_tensor(out=idxf[:], in0=idxf[:], in1=valid[:],
                                        op=Alu.mult)
                nc.vector.tensor_scalar(idxf[:], idxf[:], pad[:], scalar2=None,
                                        op0=Alu.add)
                # Wrapped, group-replicated index tensor: transpose idx to
                # [8s, 128q]; wrap rows p0<8 are even columns, p0>=8 odd
                # columns (partition starts 0 and 32 are the legal ones);
                # replicate to 8 groups via a DRAM bounce.
                tp_ps = op_pool.tile([GROUPS, 128], f32, tag="op")
                nc.tensor.transpose(tp_ps[:], idxf[:], ident[:])
                w64 = small.tile([40, 64], u16, tag="w64")
                tp3 = tp_ps[:].rearrange("p (c a) -> p a c", a=2)
                nc.vector.tensor_copy(w64[0:8, :].unsqueeze(1), tp3[:, 0:1, :])
                nc.vector.tensor_copy(w64[32:40, :].unsqueeze(1), tp3[:, 1:2, :])
                scr = drm.tile([16, 64], u16, tag="scr")
                nc.sync.dma_start(scr[0:8, :], w64[0:8, :])
                nc.sync.dma_start(scr[8:16, :], w64[32:40, :])
                wrep = small.tile([128, 64], u16, tag="wrep")
                st_wrep[i] = wrep
                nc.sync.dma_start(
                    wrep[:], scr[:].unsqueeze(0).to_broadcast([8, 16, 64]))

            def stage_c(i):
                q0 = i * 128
                wrep = st_wrep[i]
                g2c = gath.tile([128, 1024], f32, tag="g2c")
                nc.gpsimd.indirect_copy(g2c[:], f2s[:], wrep[:], True)
                feat = gath.tile([128, 1024], f32, tag="feat")
                nc.gpsimd.indirect_copy(feat[:], xyzs[:], wrep[:], True)
                # cur_corr[q,s] = sum_d f1s[d,q] * f2[d, idx[q,s]]
                f1b = f1s[:, q0:q0 + 128].unsqueeze(2).to_broadcast([128, 128, NS])
                g2c3 = g2c[:].rearrange("d (q s) -> d q s", s=NS)
                nc.vector.tensor_tensor(out=g2c3, in0=g2c3, in1=f1b, op=Alu.mult)
                cc_ps = cc_pool.tile([1, 1024], f32, tag="cc")
                for j in range(2):
                    nc.tensor.matmul(cc_ps[:, j * 512:(j + 1) * 512], ones1[:],
                                     g2c[:, j * 512:(j + 1) * 512])
                cc_sb = small.tile([1, 1024], f32, tag="ccsb")
                nc.scalar.copy(cc_sb[:], cc_ps[:])
                # conv as 3 accumulating matmuls: corr (K=1), gathered xyz
                # (K=3, rows 0-2 of the gather), bias - W123.c (K=5)
                xt_ps = xtp_pool.tile([C, 1024], f32, tag="xt")
                for j in range(2):
                    sl = slice(j * 512, (j + 1) * 512)
                    nc.tensor.matmul(xt_ps[:, sl], wcxs[:], feat[0:3, sl],
                                     start=True, stop=False)
                    nc.tensor.matmul(xt_ps[:, sl], wccs[:], cc_sb[:, sl],
                                     start=False, stop=False)
                    bq_rhs = gqs[0:5, q0 + j * 64:q0 + (j + 1) * 64]\
                        .unsqueeze(2).to_broadcast([5, 64, NS])
                    nc.tensor.matmul(xt_ps[:, sl], wcatbs[:],
                                     bq_rhs, start=False, stop=True)
                nc.scalar.activation(xtall[:, i * 1024:(i + 1) * 1024], xt_ps[:],
                                     Act.Copy, accum_out=s1c[:, i:i + 1])
                sqscr = ph2.tile([C, 1024], f32, tag="ys")
                nc.scalar.activation(sqscr[:], xt_ps[:], Act.Square,
                                     accum_out=s2c[:, i:i + 1])

            for i in range(NT + 2):
                if i < NT:
                    stage_a(i)
                if 1 <= i <= NT:
                    stage_b(i - 1)
                if i >= 2:
                    stage_c(i - 2)

            # ---------------- GroupNorm stats + pair AllReduce ----------------
            st = small.tile([C, 2], f32, tag="st")
            nc.vector.tensor_reduce(st[:, 0:1], s1c[:], axis=mybir.AxisListType.X,
                                    op=Alu.add)
            nc.vector.tensor_reduce(st[:, 1:2], s2c[:], axis=mybir.AxisListType.X,
                                    op=Alu.add)
            gs_ps = op_pool.tile([GROUPS, 2], f32, tag="op")
            nc.tensor.matmul(gs_ps[:], gsels[:], st[:])
            gss = small.tile([GROUPS, 2], f32, tag="gss")
            nc.scalar.copy(gss[:], gs_ps[:])
            cci = drm.tile([GROUPS, 2], f32, tag="cci")
            cco = drm.tile([GROUPS, 2], f32, tag="cco")
            nc.sync.dma_start(cci[:], gss[:])
            nc.gpsimd.collective_compute(
                "AllReduce", Alu.add,
                replica_groups=[[0, 1], [2, 3], [4, 5], [6, 7]],
                ins=[cci.opt()], outs=[cco.opt()])
            gsr = small.tile([GROUPS, 2], f32, tag="gsr")
            nc.sync.dma_start(gsr[:], cco[:])
            ch_ps = op_pool.tile([C, 2], f32, tag="op")
            nc.tensor.matmul(ch_ps[:], gselTs[:], gsr[:])
            chs = small.tile([C, 2], f32, tag="chs")
            nc.scalar.activation(chs[:], ch_ps[:], Act.Copy, scale=1.0 / CNT)
            # scale = gamma * rsqrt(var + eps); shift = beta - mean*scale
            mean = chs[:, 0:1]
            tmp = small.tile([C, 1], f32, tag="tmp")
            nc.vector.tensor_tensor(out=tmp[:], in0=mean, in1=mean, op=Alu.mult)
            var = small.tile([C, 1], f32, tag="var")
            nc.vector.tensor_tensor(out=var[:], in0=chs[:, 1:2], in1=tmp[:],
                                    op=Alu.subtract)
            nc.vector.tensor_scalar(var[:], var[:], EPS, scalar2=None, op0=Alu.add)
            std = small.tile([C, 1], f32, tag="std")
            nc.scalar.activation(std[:], var[:], Act.Sqrt)
            rs = small.tile([C, 1], f32, tag="rs")
            nc.vector.reciprocal(rs[:], std[:])
            gnsc = small.tile([C, 1], f32, tag="gnsc")
            nc.vector.tensor_tensor(out=gnsc[:], in0=rs[:], in1=gnabs[:, 0:1],
                                    op=Alu.mult)
            gnsh = small.tile([C, 1], f32, tag="gnsh")
            nc.vector.tensor_tensor(out=gnsh[:], in0=mean, in1=gnsc[:], op=Alu.mult)
            nc.vector.tensor_tensor(out=gnsh[:], in0=gnabs[:, 1:2], in1=gnsh[:],
                                    op=Alu.subtract)

            # ---------------- phase 2 ----------------
            for i in range(NT):
                ys = ph2.tile([C, 1024], f32, tag="ys")
                nc.scalar.activation(ys[:], xtall[:, i * 1024:(i + 1) * 1024],
                                     Act.Prelu, bias=gnsh[:], scale=gnsc[:],
                                     alpha=float(prelu_alpha))
                mp = ph2.tile([C, 128], f32, tag="mp")
                nc.vector.tensor_reduce(mp[:], ys[:].rearrange("c (q s) -> c q s", s=NS),
                                        axis=mybir.AxisListType.X, op=Alu.max)
                o_ps = op_pool.tile([C, 128], f32, tag="op")
                nc.tensor.matmul(o_ps[:], wouts[:], mp[:])
                osb = ph2.tile([C, 128], f32, tag="osb")
                nc.vector.tensor_scalar(osb[:], o_ps[:], outbs[:], scalar2=None,
                                        op0=Alu.add)
                nc.sync.dma_start(d_out[:, i * 128:(i + 1) * 128], osb[:])

    nc.finalize()
    return nc


_CACHE = {}


def kernel(coords, xyz2, fmap1, fmap2, conv_w, conv_b, gn_gamma, gn_beta,
           prelu_a, out_w, out_b):
    from concourse.bass_utils import run_bass_kernel_spmd

    coords = np.asarray(coords, np.float32)
    xyz2 = np.asarray(xyz2, np.float32)
    fmap1 = np.asarray(fmap1, np.float32)
    fmap2 = np.asarray(fmap2, np.float32)
    conv_w = np.asarray(conv_w, np.float32)
    conv_b = np.asarray(conv_b, np.float32)
    gn_gamma = np.asarray(gn_gamma, np.float32)
    gn_beta = np.asarray(gn_beta, np.float32)
    prelu_a = np.asarray(prelu_a, np.float32)
    out_w = np.asarray(out_w, np.float32)
    out_b = np.asarray(out_b, np.float32)

    alpha = float(prelu_a.reshape(-1)[0])
    key = alpha
    if key not in _CACHE:
        _CACHE[key] = _build(alpha)
    nc = _CACHE[key]

    wcc = np.ascontiguousarray(conv_w.T[0:1, :], np.float32)   # [1, 64]
    wcx = np.ascontiguousarray(conv_w.T[1:4, :], np.float32)   # [3, 64]
    wcatb = np.concatenate([-0.5 * conv_w[:, 1:4].T, np.zeros((1, C), np.float32),
                            conv_b[None, :]], axis=0).astype(np.float32)  # [5, 64]
    wout = np.ascontiguousarray(out_w.T, np.float32)       # [64, 64]
    gnab = np.stack([gn_gamma, gn_beta], axis=1)           # [64, 2]
    outb = np.ascontiguousarray(out_b[:, None], np.float32)
    gsel = np.zeros((C, GROUPS), np.float32)
    gsel[np.arange(C), np.arange(C) // (C // GROUPS)] = 1.0
    gselT = np.ascontiguousarray(gsel.T)
    ones1 = np.ones((128, 1), np.float32)
    ident = np.eye(128, dtype=np.float32)

    in_maps = []
    for core in range(8):
        b, h = core // 2, core % 2
        qsl = slice(h * HALF, (h + 1) * HALF)
        cq3 = coords[b, qsl].astype(np.float64)            # [2048, 3]
        xyz = xyz2[b].astype(np.float64)                   # [4096, 3]
        gq = np.stack([2.0 * cq3[:, 0], 2.0 * cq3[:, 1], 2.0 * cq3[:, 2],
                       RADIUS * RADIUS - np.sum(cq3 * cq3, axis=1),
                       np.ones(HALF)], axis=0).astype(np.float32)
        gp = np.stack([xyz[:, 0], xyz[:, 1], xyz[:, 2], np.ones(N),
                       -np.sum(xyz * xyz, axis=1)], axis=0).astype(np.float32)
        f1sc = np.ascontiguousarray(
            fmap1[b][:, qsl] * np.float32(1.0 / np.sqrt(DIM)), np.float32)
        xyzd = np.concatenate([np.ascontiguousarray(xyz2[b].T),
                               np.zeros((1, N), np.float32)], axis=0)
        in_maps.append({
            "gq": gq, "gp": gp, "f1s": f1sc,
            "f2": np.ascontiguousarray(fmap2[b]), "xyzd": xyzd,
            "wcc": wcc, "wcx": wcx, "wcatb": wcatb, "wout": wout, "gnab": gnab, "outb": outb,
            "gsel": gsel, "gselT": gselT, "ones1": ones1, "ident": ident,
        })

    kres = run_bass_kernel_spmd(nc, in_maps, list(range(8)),
                                trace=bool(globals().get("TRACE", False)))
    globals()["LAST_RESULTS"] = kres
    res = kres.results

    out = np.empty((B, C, N), np.float32)
    for core in range(8):
        b, h = core // 2, core % 2
        out[b, :, h * HALF:(h + 1) * HALF] = res[core]["out"]
    return out


# revision 12
# speedup vs baseline: 1.0065x; 1.0065x over previous
"""Trainium2 Bass kernel for nn_BQ_CorrBlock (ball-query + corr gather + conv/GN/PReLU/max/conv).

Sharding: 8 cores = (batch b in 0..3) x (query-half h in 0..1). Each core handles
2048 queries x 4096 points. GroupNorm statistics span a full batch item, so the
two cores of a pair AllReduce their per-group partial sums (16 floats).

Algorithm per core (16 query tiles of 128):
  - PE computes g[q,p] = r^2 - |c|^2 + 2 c.x - |x|^2  (positive iff inside ball)
    via a K=5 matmul against host-prepared augmented coordinates.
  - score = (g > 0) * (N - p)  in one fused scalar_tensor_tensor; DVE max8 gives
    the 8 smallest valid point indices per query (PointNet++ ball query order).
  - Indices round-trip through DRAM to build the 16-partition-wrapped, group-
    replicated index tensor that gpsimd indirect_copy expects; two gathers pull
    f2 columns [128d x 1024] and xyz2 components for all (q,s) pairs.
  - cur_corr = ones^T (f1_scaled * f2cols) via elementwise mult + K=128 matmul.
  - 1x1 conv (+ bias - W123.c folded in) as one K=8 matmul -> x[64, (q,s)].
  - GroupNorm stats accumulated via activation accum_out; pair AllReduce;
    normalize+PReLU in one activation; max over s; final 64x64 matmul + bias.
"""

import os
import sys

import numpy as np

for _p in ("/opt/trn_rl_repo", "/root/.axon_site/_ro/trn_rl_repo"):
    if os.path.isdir(_p) and _p not in sys.path:
        sys.path.append(_p)

RADIUS = 1.0
NS = 8
GROUPS = 8
EPS = 1e-5
B, N, DIM, C = 4, 4096, 128, 64
HALF = N // 2          # queries per core
NT = HALF // 128       # query tiles per core
CNT = (C // GROUPS) * N * NS  # elements per (batch, group) for GN stats


def _build(prelu_alpha: float):
    import concourse.bacc as bacc
    import concourse.mybir as mybir
    import concourse.tile as tile

    f32 = mybir.dt.float32
    u16 = mybir.dt.uint16
    Alu = mybir.AluOpType
    Act = mybir.ActivationFunctionType

    nc = bacc.Bacc(None)

    bf16 = mybir.dt.bfloat16
    d_gq = nc.dram_tensor("gq", [45, HALF], bf16, kind="ExternalInput")
    d_gp = nc.dram_tensor("gp", [45, N], bf16, kind="ExternalInput")
    d_cq = nc.dram_tensor("cq", [4, HALF], f32, kind="ExternalInput")
    d_f1 = nc.dram_tensor("f1s", [128, HALF], f32, kind="ExternalInput")
    d_f2 = nc.dram_tensor("f2", [128, N], f32, kind="ExternalInput")
    d_xyz = nc.dram_tensor("xyzd", [4, N], f32, kind="ExternalInput")
    d_wcc = nc.dram_tensor("wcc", [1, C], f32, kind="ExternalInput")
    d_wcx = nc.dram_tensor("wcx", [3, C], f32, kind="ExternalInput")
    d_wcatb = nc.dram_tensor("wcatb", [4, C], f32, kind="ExternalInput")
    d_wout = nc.dram_tensor("wout", [C, C], f32, kind="ExternalInput")
    d_gnab = nc.dram_tensor("gnab", [C, 2], f32, kind="ExternalInput")
    d_outb = nc.dram_tensor("outb", [C, 1], f32, kind="ExternalInput")
    d_gsel = nc.dram_tensor("gsel", [C, GROUPS], f32, kind="ExternalInput")
    d_gselT = nc.dram_tensor("gselT", [GROUPS, C], f32, kind="ExternalInput")
    d_ones = nc.dram_tensor("ones1", [128, 1], f32, kind="ExternalInput")
    d_ident = nc.dram_tensor("ident", [128, 128], f32, kind="ExternalInput")
    d_out = nc.dram_tensor("out", [C, HALF], f32, kind="ExternalOutput")

    with tile.TileContext(nc) as tc:
        with (
            tc.tile_pool(name="cst", bufs=1) as cst,
            tc.tile_pool(name="score", bufs=1) as score_pool,
            tc.tile_pool(name="small", bufs=2) as small,
            tc.tile_pool(name="gath", bufs=3) as gath,
            tc.tile_pool(name="xtp", bufs=1) as xt_pool,
            tc.tile_pool(name="ph2", bufs=2) as ph2,
            tc.tile_pool(name="gps", bufs=3, space="PSUM") as gpsum_pool,
            tc.tile_pool(name="ccp", bufs=1, space="PSUM") as cc_pool,
            tc.tile_pool(name="xtps", bufs=1, space="PSUM") as xtp_pool,
            tc.tile_pool(name="ops", bufs=1, space="PSUM") as op_pool,
            tc.tile_pool(name="drm", bufs=2, space="DRAM") as drm,
        ):
            # ---- resident constants ----
            gqs = cst.tile([45, HALF], bf16)
            gps = cst.tile([45, N], bf16)
            cqs = cst.tile([4, HALF], f32)
            f1s = cst.tile([128, HALF], f32)
            f2s = cst.tile([128, N], f32)
            xyzs = cst.tile([128, N], f32)
            wccs = cst.tile([1, C], f32)
            wcxs = cst.tile([3, C], f32)
            wcatbs = cst.tile([4, C], f32)
            wouts = cst.tile([C, C], f32)
            gnabs = cst.tile([C, 2], f32)
            outbs = cst.tile([C, 1], f32)
            gsels = cst.tile([C, GROUPS], f32)
            gselTs = cst.tile([GROUPS, C], f32)
            ones1 = cst.tile([128, 1], f32)
            ident = cst.tile([128, 128], f32)
            nmj = cst.tile([128, N], mybir.dt.int16)

            for t, d in [(gqs, d_gq), (gps, d_gp), (cqs, d_cq), (f1s, d_f1),
                         (f2s, d_f2), (wccs, d_wcc), (wcxs, d_wcx), (wcatbs, d_wcatb), (wouts, d_wout),
                         (gnabs, d_gnab), (outbs, d_outb), (gsels, d_gsel),
                         (gselTs, d_gselT), (ones1, d_ones), (ident, d_ident)]:
                nc.sync.dma_start(t[:], d[:])
            nc.gpsimd.memset(xyzs[:], 0)
            nc.sync.dma_start(xyzs[0:4, :], d_xyz[:])
            # nmj[p, j] = N - j (same on every partition); exact in fp32
            nc.gpsimd.iota(nmj[:], pattern=[[-1, N]], base=N, channel_multiplier=0,
                           allow_small_or_imprecise_dtypes=True)

            s1c = cst.tile([C, NT], f32)   # per-tile sums of x
            s2c = cst.tile([C, NT], f32)   # per-tile sums of x^2
            xtall = cst.tile([C, NT * 1024], f32)  # conv outputs, resident

            # ---------------- phase 1 (3-stage software pipeline) ----------------
            # Stage A(i): all-pairs metric + score.  Stage B(i): top-8 + index
            # math + wrapped-index build (PE transpose, DMA bounce).  Stage
            # C(i): gathers + cur_corr + conv + stats.  Emitting A(i), B(i-1),
            # C(i-2) per iteration gives each cross-engine hop a full stage of
            # slack, so the DMA/gather latency hides behind the next tile's
            # score work.
            st_score = [None] * NT
            st_idxf = [None] * NT
            st_wrep = [None] * NT

            def stage_a(i):
                q0 = i * 128
                score = score_pool.tile([128, N], f32, tag="score")
                st_score[i] = score
                for s in range(8):
                    gp_ps = gpsum_pool.tile([128, 512], f32, tag="gps")
                    nc.tensor.matmul(gp_ps[:], gqs[:, q0:q0 + 128],
                                     gps[:, s * 512:(s + 1) * 512])
                    nc.vector.scalar_tensor_tensor(
                        score[:, s * 512:(s + 1) * 512], gp_ps[:], 0.0,
                        nmj[:, s * 512:(s + 1) * 512],
                        op0=Alu.is_gt, op1=Alu.mult)

            def stage_b(i):
                score = st_score[i]
                s8 = small.tile([128, NS], f32, tag="s8")
                nc.vector.max(out=s8[:], in_=score[:])
                valid = small.tile([128, NS], f32, tag="valid")
                nc.vector.tensor_scalar(valid[:], s8[:], 0.5, scalar2=None,
                                        op0=Alu.is_gt)
                idxf = small.tile([128, NS], f32, tag="idxf")
                st_idxf[i] = idxf
                nc.vector.tensor_scalar(idxf[:], s8[:], -1.0, scalar2=float(N),
                                        op0=Alu.mult, op1=Alu.add)
                nc.vector.tensor_tensor(out=idxf[:], in0=idxf[:], in1=valid[:],
                                        op=Alu.mult)
                pad = small.tile([128, 1], f32, tag="pad")
                nc.vector.tensor_copy(pad[:], idxf[:, 0:1])
                # idx = valid ? idxf : pad  ==  pad + valid*(idxf - pad)
                nc.vector.tensor_scalar(idxf[:], idxf[:], pad[:], scalar2=None,
                                        op0=Alu.subtract)
                nc.vector.tensor_tensor(out=idxf[:], in0=idxf[:], in1=valid[:],
                                        op=Alu.mult)
                nc.vector.tensor_scalar(idxf[:], idxf[:], pad[:], scalar2=None,
                                        op0=Alu.add)
                # Wrapped, group-replicated index tensor: transpose idx to
                # [8s, 128q]; wrap rows p0<8 are even columns, p0>=8 odd
                # columns (partition starts 0 and 32 are the legal ones);
                # replicate to 8 groups via a DRAM bounce.
                tp_ps = op_pool.tile([GROUPS, 128], f32, tag="op")
                nc.tensor.transpose(tp_ps[:], idxf[:], ident[:])
                w64 = small.tile([40, 64], u16, tag="w64")
                tp3 = tp_ps[:].rearrange("p (c a) -> p a c", a=2)
                nc.vector.tensor_copy(w64[0:8, :].unsqueeze(1), tp3[:, 0:1, :])
                nc.vector.tensor_copy(w64[32:40, :].unsqueeze(1), tp3[:, 1:2, :])
                scr = drm.tile([16, 64], u16, tag="scr")
                nc.sync.dma_start(scr[0:8, :], w64[0:8, :])
                nc.sync.dma_start(scr[8:16, :], w64[32:40, :])
                wrep = small.tile([128, 64], u16, tag="wrep")
                st_wrep[i] = wrep
                nc.sync.dma_start(
                    wrep[:], scr[:].unsqueeze(0).to_broadcast([8, 16, 64]))

            def stage_c(i):
                q0 = i * 128
                wrep = st_wrep[i]
                g2c = gath.tile([128, 1024], f32, tag="g2c")
                nc.gpsimd.indirect_copy(g2c[:], f2s[:], wrep[:], True)
                feat = gath.tile([128, 1024], f32, tag="feat")
                nc.gpsimd.indirect_copy(feat[:], xyzs[:], wrep[:], True)
                # cur_corr[q,s] = sum_d f1s[d,q] * f2[d, idx[q,s]]
                f1b = f1s[:, q0:q0 + 128].unsqueeze(2).to_broadcast([128, 128, NS])
                g2c3 = g2c[:].rearrange("d (q s) -> d q s", s=NS)
                nc.vector.tensor_tensor(out=g2c3, in0=g2c3, in1=f1b, op=Alu.mult)
                cc_ps = cc_pool.tile([1, 1024], f32, tag="cc")
                for j in range(2):
                    nc.tensor.matmul(cc_ps[:, j * 512:(j + 1) * 512], ones1[:],
                                     g2c[:, j * 512:(j + 1) * 512])
                cc_sb = small.tile([1, 1024], f32, tag="ccsb")
                nc.scalar.copy(cc_sb[:], cc_ps[:])
                # conv as 3 accumulating matmuls: corr (K=1), gathered xyz
                # (K=3, rows 0-2 of the gather), bias - W123.c (K=5)
                xt_ps = xtp_pool.tile([C, 1024], f32, tag="xt")
                for j in range(2):
                    sl = slice(j * 512, (j + 1) * 512)
                    nc.tensor.matmul(xt_ps[:, sl], wcxs[:], feat[0:3, sl],
                                     start=True, stop=False)
                    nc.tensor.matmul(xt_ps[:, sl], wccs[:], cc_sb[:, sl],
                                     start=False, stop=False)
                    bq_rhs = cqs[0:4, q0 + j * 64:q0 + (j + 1) * 64]\
                        .unsqueeze(2).to_broadcast([4, 64, NS])
                    nc.tensor.matmul(xt_ps[:, sl], wcatbs[:],
                                     bq_rhs, start=False, stop=True)
                nc.scalar.activation(xtall[:, i * 1024:(i + 1) * 1024], xt_ps[:],
                                     Act.Copy, accum_out=s1c[:, i:i + 1])
                sqscr = ph2.tile([C, 1024], f32, tag="ys")
                nc.scalar.activation(sqscr[:], xt_ps[:], Act.Square,
                                     accum_out=s2c[:, i:i + 1])

            for i in range(NT + 2):
                if i < NT:
                    stage_a(i)
                if 1 <= i <= NT:
                    stage_b(i - 1)
                if i >= 2:
                    stage_c(i - 2)

            # ---------------- GroupNorm stats + pair AllReduce ----------------
            st = small.tile([C, 2], f32, tag="st")
            nc.vector.tensor_reduce(st[:, 0:1], s1c[:], axis=mybir.AxisListType.X,
                                    op=Alu.add)
            nc.vector.tensor_reduce(st[:, 1:2], s2c[:], axis=mybir.AxisListType.X,
                                    op=Alu.add)
            gs_ps = op_pool.tile([GROUPS, 2], f32, tag="op")
            nc.tensor.matmul(gs_ps[:], gsels[:], st[:])
            gss = small.tile([GROUPS, 2], f32, tag="gss")
            nc.scalar.copy(gss[:], gs_ps[:])
            cci = drm.tile([GROUPS, 2], f32, tag="cci")
            cco = drm.tile([GROUPS, 2], f32, tag="cco")
            nc.sync.dma_start(cci[:], gss[:])
            nc.gpsimd.collective_compute(
                "AllReduce", Alu.add,
                replica_groups=[[0, 1], [2, 3], [4, 5], [6, 7]],
                ins=[cci.opt()], outs=[cco.opt()])
            gsr = small.tile([GROUPS, 2], f32, tag="gsr")
            nc.sync.dma_start(gsr[:], cco[:])
            ch_ps = op_pool.tile([C, 2], f32, tag="op")
            nc.tensor.matmul(ch_ps[:], gselTs[:], gsr[:])
            chs = small.tile([C, 2], f32, tag="chs")
            nc.scalar.activation(chs[:], ch_ps[:], Act.Copy, scale=1.0 / CNT)
            # scale = gamma * rsqrt(var + eps); shift = beta - mean*scale
            mean = chs[:, 0:1]
            tmp = small.tile([C, 1], f32, tag="tmp")
            nc.vector.tensor_tensor(out=tmp[:], in0=mean, in1=mean, op=Alu.mult)
            var = small.tile([C, 1], f32, tag="var")
            nc.vector.tensor_tensor(out=var[:], in0=chs[:, 1:2], in1=tmp[:],
                                    op=Alu.subtract)
            nc.vector.tensor_scalar(var[:], var[:], EPS, scalar2=None, op0=Alu.add)
            std = small.tile([C, 1], f32, tag="std")
            nc.scalar.activation(std[:], var[:], Act.Sqrt)
            rs = small.tile([C, 1], f32, tag="rs")
            nc.vector.reciprocal(rs[:], std[:])
            gnsc = small.tile([C, 1], f32, tag="gnsc")
            nc.vector.tensor_tensor(out=gnsc[:], in0=rs[:], in1=gnabs[:, 0:1],
                                    op=Alu.mult)
            gnsh = small.tile([C, 1], f32, tag="gnsh")
            nc.vector.tensor_tensor(out=gnsh[:], in0=mean, in1=gnsc[:], op=Alu.mult)
            nc.vector.tensor_tensor(out=gnsh[:], in0=gnabs[:, 1:2], in1=gnsh[:],
                                    op=Alu.subtract)

            # ---------------- phase 2 ----------------
            for i in range(NT):
                ys = ph2.tile([C, 1024], f32, tag="ys")
                nc.scalar.activation(ys[:], xtall[:, i * 1024:(i + 1) * 1024],
                                     Act.Prelu, bias=gnsh[:], scale=gnsc[:],
                                     alpha=float(prelu_alpha))
                mp = ph2.tile([C, 128], f32, tag="mp")
                nc.vector.tensor_reduce(mp[:], ys[:].rearrange("c (q s) -> c q s", s=NS),
                                        axis=mybir.AxisListType.X, op=Alu.max)
                o_ps = op_pool.tile([C, 128], f32, tag="op")
                nc.tensor.matmul(o_ps[:], wouts[:], mp[:])
                osb = ph2.tile([C, 128], f32, tag="osb")
                nc.vector.tensor_scalar(osb[:], o_ps[:], outbs[:], scalar2=None,
                                        op0=Alu.add)
                nc.sync.dma_start(d_out[:, i * 128:(i + 1) * 128], osb[:])

    nc.finalize()
    return nc


_CACHE = {}


def _bf16_split_pair(a5, b5):
    """3-level bf16 split of both operands of the g-matmul; all 9 level pairs
    stacked into K=45 so one bf16 matmul reproduces the fp32 product to
    ~2^-26 relative."""
    import ml_dtypes
    bf = ml_dtypes.bfloat16

    def split(v):
        h = v.astype(bf)
        m = (v - h.astype(np.float64)).astype(bf)
        l = (v - h.astype(np.float64) - m.astype(np.float64)).astype(bf)
        return h, m, l

    ah, am, al = split(a5)
    bh, bm, bl = split(b5)
    pairs = [(ah, bh), (ah, bm), (am, bh), (ah, bl), (al, bh),
             (am, bm), (am, bl), (al, bm), (al, bl)]
    a45 = np.concatenate([p[0] for p in pairs], axis=0)
    b45 = np.concatenate([p[1] for p in pairs], axis=0)
    return np.ascontiguousarray(a45), np.ascontiguousarray(b45)


def kernel(coords, xyz2, fmap1, fmap2, conv_w, conv_b, gn_gamma, gn_beta,
           prelu_a, out_w, out_b):
    from concourse.bass_utils import run_bass_kernel_spmd

    coords = np.asarray(coords, np.float32)
    xyz2 = np.asarray(xyz2, np.float32)
    fmap1 = np.asarray(fmap1, np.float32)
    fmap2 = np.asarray(fmap2, np.float32)
    conv_w = np.asarray(conv_w, np.float32)
    conv_b = np.asarray(conv_b, np.float32)
    gn_gamma = np.asarray(gn_gamma, np.float32)
    gn_beta = np.asarray(gn_beta, np.float32)
    prelu_a = np.asarray(prelu_a, np.float32)
    out_w = np.asarray(out_w, np.float32)
    out_b = np.asarray(out_b, np.float32)

    alpha = float(prelu_a.reshape(-1)[0])
    key = alpha
    if key not in _CACHE:
        _CACHE[key] = _build(alpha)
    nc = _CACHE[key]

    wcc = np.ascontiguousarray(conv_w.T[0:1, :], np.float32)   # [1, 64]
    wcx = np.ascontiguousarray(conv_w.T[1:4, :], np.float32)   # [3, 64]
    wcatb = np.concatenate([-conv_w[:, 1:4].T,
                            conv_b[None, :]], axis=0).astype(np.float32)  # [4, 64]
    wout = np.ascontiguousarray(out_w.T, np.float32)       # [64, 64]
    gnab = np.stack([gn_gamma, gn_beta], axis=1)           # [64, 2]
    outb = np.ascontiguousarray(out_b[:, None], np.float32)
    gsel = np.zeros((C, GROUPS), np.float32)
    gsel[np.arange(C), np.arange(C) // (C // GROUPS)] = 1.0
    gselT = np.ascontiguousarray(gsel.T)
    ones1 = np.ones((128, 1), np.float32)
    ident = np.eye(128, dtype=np.float32)

    in_maps = []
    for core in range(8):
        b, h = core // 2, core % 2
        qsl = slice(h * HALF, (h + 1) * HALF)
        cq3 = coords[b, qsl].astype(np.float64)            # [2048, 3]
        xyz = xyz2[b].astype(np.float64)                   # [4096, 3]
        gq5 = np.stack([2.0 * cq3[:, 0], 2.0 * cq3[:, 1], 2.0 * cq3[:, 2],
                        RADIUS * RADIUS - np.sum(cq3 * cq3, axis=1),
                        np.ones(HALF)], axis=0)
        cq = np.stack([cq3[:, 0], cq3[:, 1], cq3[:, 2], np.ones(HALF)],
                      axis=0).astype(np.float32)
        gp5 = np.stack([xyz[:, 0], xyz[:, 1], xyz[:, 2], np.ones(N),
                        -np.sum(xyz * xyz, axis=1)], axis=0)
        gq, gp = _bf16_split_pair(gq5, gp5)
        f1sc = np.ascontiguousarray(
            fmap1[b][:, qsl] * np.float32(1.0 / np.sqrt(DIM)), np.float32)
        xyzd = np.concatenate([np.ascontiguousarray(xyz2[b].T),
                               np.zeros((1, N), np.float32)], axis=0)
        in_maps.append({
            "gq": gq, "gp": gp, "cq": cq, "f1s": f1sc,
            "f2": np.ascontiguousarray(fmap2[b]), "xyzd": xyzd,
            "wcc": wcc, "wcx": wcx, "wcatb": wcatb, "wout": wout, "gnab": gnab, "outb": outb,
            "gsel": gsel, "gselT": gselT, "ones1": ones1, "ident": ident,
        })

    kres = run_bass_kernel_spmd(nc, in_maps, list(range(8)),
                                trace=bool(globals().get("TRACE", False)))
    globals()["LAST_RESULTS"] = kres
    res = kres.results

    out = np.empty((B, C, N), np.float32)
    for core in range(8):
        b, h = core // 2, core % 2
        out[b, :, h * HALF:(h + 1) * HALF] = res[core]["out"]
    return out


# revision 13
# speedup vs baseline: 1.0175x; 1.0109x over previous
"""Trainium2 Bass kernel for nn_BQ_CorrBlock (ball-query + corr gather + conv/GN/PReLU/max/conv).

Sharding: 8 cores = (batch b in 0..3) x (query-half h in 0..1). Each core handles
2048 queries x 4096 points. GroupNorm statistics span a full batch item, so the
two cores of a pair AllReduce their per-group partial sums (16 floats).

Algorithm per core (16 query tiles of 128):
  - PE computes g[q,p] = r^2 - |c|^2 + 2 c.x - |x|^2  (positive iff inside ball)
    via a K=5 matmul against host-prepared augmented coordinates.
  - score = (g > 0) * (N - p)  in one fused scalar_tensor_tensor; DVE max8 gives
    the 8 smallest valid point indices per query (PointNet++ ball query order).
  - Indices round-trip through DRAM to build the 16-partition-wrapped, group-
    replicated index tensor that gpsimd indirect_copy expects; two gathers pull
    f2 columns [128d x 1024] and xyz2 components for all (q,s) pairs.
  - cur_corr = ones^T (f1_scaled * f2cols) via elementwise mult + K=128 matmul.
  - 1x1 conv (+ bias - W123.c folded in) as one K=8 matmul -> x[64, (q,s)].
  - GroupNorm stats accumulated via activation accum_out; pair AllReduce;
    normalize+PReLU in one activation; max over s; final 64x64 matmul + bias.
"""

import os
import sys

import numpy as np

for _p in ("/opt/trn_rl_repo", "/root/.axon_site/_ro/trn_rl_repo"):
    if os.path.isdir(_p) and _p not in sys.path:
        sys.path.append(_p)

RADIUS = 1.0
NS = 8
GROUPS = 8
EPS = 1e-5
B, N, DIM, C = 4, 4096, 128, 64
HALF = N // 2          # queries per core
NT = HALF // 128       # query tiles per core
CNT = (C // GROUPS) * N * NS  # elements per (batch, group) for GN stats


def _build(prelu_alpha: float):
    import concourse.bacc as bacc
    import concourse.mybir as mybir
    import concourse.tile as tile

    f32 = mybir.dt.float32
    u16 = mybir.dt.uint16
    Alu = mybir.AluOpType
    Act = mybir.ActivationFunctionType

    nc = bacc.Bacc(None)

    bf16 = mybir.dt.bfloat16
    d_gq = nc.dram_tensor("gq", [45, HALF], bf16, kind="ExternalInput")
    d_gp = nc.dram_tensor("gp", [45, N], bf16, kind="ExternalInput")
    d_cq = nc.dram_tensor("cq", [4, HALF], f32, kind="ExternalInput")
    d_f1 = nc.dram_tensor("f1s", [128, HALF], f32, kind="ExternalInput")
    d_f2 = nc.dram_tensor("f2", [128, N], f32, kind="ExternalInput")
    d_xyz = nc.dram_tensor("xyzd", [4, N], f32, kind="ExternalInput")
    d_wcc = nc.dram_tensor("wcc", [1, C], f32, kind="ExternalInput")
    d_wcx = nc.dram_tensor("wcx", [3, C], f32, kind="ExternalInput")
    d_wcatb = nc.dram_tensor("wcatb", [4, C], f32, kind="ExternalInput")
    d_wout = nc.dram_tensor("wout", [C, C], f32, kind="ExternalInput")
    d_gnab = nc.dram_tensor("gnab", [C, 2], f32, kind="ExternalInput")
    d_outb = nc.dram_tensor("outb", [C, 1], f32, kind="ExternalInput")
    d_gsel = nc.dram_tensor("gsel", [C, GROUPS], f32, kind="ExternalInput")
    d_gselT = nc.dram_tensor("gselT", [GROUPS, C], f32, kind="ExternalInput")
    d_ones = nc.dram_tensor("ones1", [128, 1], f32, kind="ExternalInput")
    d_ident = nc.dram_tensor("ident", [128, 128], f32, kind="ExternalInput")
    d_out = nc.dram_tensor("out", [C, HALF], f32, kind="ExternalOutput")

    with tile.TileContext(nc) as tc:
        with (
            tc.tile_pool(name="cst", bufs=1) as cst,
            tc.tile_pool(name="score", bufs=1) as score_pool,
            tc.tile_pool(name="small", bufs=2) as small,
            tc.tile_pool(name="gath", bufs=3) as gath,
            tc.tile_pool(name="xtp", bufs=1) as xt_pool,
            tc.tile_pool(name="ph2", bufs=2) as ph2,
            tc.tile_pool(name="gps", bufs=3, space="PSUM") as gpsum_pool,
            tc.tile_pool(name="ccp", bufs=1, space="PSUM") as cc_pool,
            tc.tile_pool(name="xtps", bufs=1, space="PSUM") as xtp_pool,
            tc.tile_pool(name="ops", bufs=1, space="PSUM") as op_pool,
            tc.tile_pool(name="drm", bufs=2, space="DRAM") as drm,
        ):
            # ---- resident constants ----
            gqs = cst.tile([45, HALF], bf16)
            gps = cst.tile([45, N], bf16)
            cqs = cst.tile([4, HALF], f32)
            f1s = cst.tile([128, HALF], f32)
            f2s = cst.tile([128, N], f32)
            xyzs = cst.tile([128, N], f32)
            wccs = cst.tile([1, C], f32)
            wcxs = cst.tile([3, C], f32)
            wcatbs = cst.tile([4, C], f32)
            wouts = cst.tile([C, C], f32)
            gnabs = cst.tile([C, 2], f32)
            outbs = cst.tile([C, 1], f32)
            gsels = cst.tile([C, GROUPS], f32)
            gselTs = cst.tile([GROUPS, C], f32)
            ones1 = cst.tile([128, 1], f32)
            ident = cst.tile([128, 128], f32)
            nmj = cst.tile([128, N], mybir.dt.int16)

            for t, d in [(gqs, d_gq), (gps, d_gp), (cqs, d_cq), (f1s, d_f1),
                         (f2s, d_f2), (wccs, d_wcc), (wcxs, d_wcx), (wcatbs, d_wcatb), (wouts, d_wout),
                         (gnabs, d_gnab), (outbs, d_outb), (gsels, d_gsel),
                         (gselTs, d_gselT), (ones1, d_ones), (ident, d_ident)]:
                nc.sync.dma_start(t[:], d[:])
            nc.gpsimd.memset(xyzs[:], 0)
            nc.sync.dma_start(xyzs[0:4, :], d_xyz[:])
            # nmj[p, j] = N - j (same on every partition); exact in fp32
            nc.gpsimd.iota(nmj[:], pattern=[[-1, N]], base=N, channel_multiplier=0,
                           allow_small_or_imprecise_dtypes=True)

            s1c = cst.tile([C, NT], f32)   # per-tile sums of x
            s2c = cst.tile([C, NT], f32)   # per-tile sums of x^2
            xtall = cst.tile([C, NT * 1024], f32)  # conv outputs, resident

            # ---------------- phase 1 (3-stage software pipeline) ----------------
            # Stage A(i): all-pairs metric + score.  Stage B(i): top-8 + index
            # math + wrapped-index build (PE transpose, DMA bounce).  Stage
            # C(i): gathers + cur_corr + conv + stats.  Emitting A(i), B(i-1),
            # C(i-2) per iteration gives each cross-engine hop a full stage of
            # slack, so the DMA/gather latency hides behind the next tile's
            # score work.
            st_score = [None] * NT
            st_idxf = [None] * NT
            st_wrep = [None] * NT

            def stage_a(i):
                q0 = i * 128
                score = score_pool.tile([128, N], f32, tag="score")
                st_score[i] = score
                for s in range(8):
                    gp_ps = gpsum_pool.tile([128, 512], f32, tag="gps")
                    nc.tensor.matmul(gp_ps[:], gqs[:, q0:q0 + 128],
                                     gps[:, s * 512:(s + 1) * 512])
                    nc.vector.scalar_tensor_tensor(
                        score[:, s * 512:(s + 1) * 512], gp_ps[:], 0.0,
                        nmj[:, s * 512:(s + 1) * 512],
                        op0=Alu.is_gt, op1=Alu.mult)

            def stage_b(i):
                score = st_score[i]
                s8 = small.tile([128, NS], f32, tag="s8")
                nc.vector.max(out=s8[:], in_=score[:])
                valid = small.tile([128, NS], f32, tag="valid")
                nc.vector.tensor_scalar(valid[:], s8[:], 0.5, scalar2=None,
                                        op0=Alu.is_gt)
                idxf = small.tile([128, NS], f32, tag="idxf")
                st_idxf[i] = idxf
                nc.vector.tensor_scalar(idxf[:], s8[:], -1.0, scalar2=float(N),
                                        op0=Alu.mult, op1=Alu.add)
                nc.vector.tensor_tensor(out=idxf[:], in0=idxf[:], in1=valid[:],
                                        op=Alu.mult)
                pad = small.tile([128, 1], f32, tag="pad")
                nc.vector.tensor_copy(pad[:], idxf[:, 0:1])
                # idx = valid ? idxf : pad  ==  pad + valid*(idxf - pad)
                nc.vector.tensor_scalar(idxf[:], idxf[:], pad[:], scalar2=None,
                                        op0=Alu.subtract)
                nc.vector.tensor_tensor(out=idxf[:], in0=idxf[:], in1=valid[:],
                                        op=Alu.mult)
                nc.vector.tensor_scalar(idxf[:], idxf[:], pad[:], scalar2=None,
                                        op0=Alu.add)
                # Wrapped, group-replicated index tensor: transpose idx to
                # [8s, 128q]; wrap rows p0<8 are even columns, p0>=8 odd
                # columns (partition starts 0 and 32 are the legal ones);
                # replicate to 8 groups via a DRAM bounce.
                tp_ps = op_pool.tile([GROUPS, 128], f32, tag="op")
                nc.tensor.transpose(tp_ps[:], idxf[:], ident[:])
                w64 = small.tile([40, 64], u16, tag="w64")
                tp3 = tp_ps[:].rearrange("p (c a) -> p a c", a=2)
                nc.vector.tensor_copy(w64[0:8, :].unsqueeze(1), tp3[:, 0:1, :])
                nc.vector.tensor_copy(w64[32:40, :].unsqueeze(1), tp3[:, 1:2, :])
                scr = drm.tile([16, 64], u16, tag="scr")
                nc.sync.dma_start(scr[0:8, :], w64[0:8, :])
                nc.sync.dma_start(scr[8:16, :], w64[32:40, :])
                wrep = small.tile([128, 64], u16, tag="wrep")
                st_wrep[i] = wrep
                nc.sync.dma_start(
                    wrep[:], scr[:].unsqueeze(0).to_broadcast([8, 16, 64]))

            st_g2c = [None] * NT
            st_feat = [None] * NT

            def stage_c1(i):
                wrep = st_wrep[i]
                g2c = gath.tile([128, 1024], f32, tag="g2c")
                st_g2c[i] = g2c
                nc.gpsimd.indirect_copy(g2c[:], f2s[:], wrep[:], True)
                feat = gath.tile([128, 1024], f32, tag="feat")
                st_feat[i] = feat
                nc.gpsimd.indirect_copy(feat[:], xyzs[:], wrep[:], True)

            def stage_c2(i):
                q0 = i * 128
                g2c = st_g2c[i]
                feat = st_feat[i]
                # cur_corr[q,s] = sum_d f1s[d,q] * f2[d, idx[q,s]]
                f1b = f1s[:, q0:q0 + 128].unsqueeze(2).to_broadcast([128, 128, NS])
                g2c3 = g2c[:].rearrange("d (q s) -> d q s", s=NS)
                nc.vector.tensor_tensor(out=g2c3, in0=g2c3, in1=f1b, op=Alu.mult)
                cc_ps = cc_pool.tile([1, 1024], f32, tag="cc")
                for j in range(2):
                    nc.tensor.matmul(cc_ps[:, j * 512:(j + 1) * 512], ones1[:],
                                     g2c[:, j * 512:(j + 1) * 512])
                cc_sb = small.tile([1, 1024], f32, tag="ccsb")
                nc.scalar.copy(cc_sb[:], cc_ps[:])
                # conv as 3 accumulating matmuls: corr (K=1), gathered xyz
                # (K=3, rows 0-2 of the gather), bias - W123.c (K=5)
                xt_ps = xtp_pool.tile([C, 1024], f32, tag="xt")
                for j in range(2):
                    sl = slice(j * 512, (j + 1) * 512)
                    nc.tensor.matmul(xt_ps[:, sl], wcxs[:], feat[0:3, sl],
                                     start=True, stop=False)
                    nc.tensor.matmul(xt_ps[:, sl], wccs[:], cc_sb[:, sl],
                                     start=False, stop=False)
                    bq_rhs = cqs[0:4, q0 + j * 64:q0 + (j + 1) * 64]\
                        .unsqueeze(2).to_broadcast([4, 64, NS])
                    nc.tensor.matmul(xt_ps[:, sl], wcatbs[:],
                                     bq_rhs, start=False, stop=True)
                nc.scalar.activation(xtall[:, i * 1024:(i + 1) * 1024], xt_ps[:],
                                     Act.Copy, accum_out=s1c[:, i:i + 1])
                sqscr = ph2.tile([C, 1024], f32, tag="ys")
                nc.scalar.activation(sqscr[:], xt_ps[:], Act.Square,
                                     accum_out=s2c[:, i:i + 1])

            for i in range(NT + 3):
                if i < NT:
                    stage_a(i)
                if 1 <= i <= NT:
                    stage_b(i - 1)
                if 2 <= i <= NT + 1:
                    stage_c1(i - 2)
                if i >= 3:
                    stage_c2(i - 3)

            # ---------------- GroupNorm stats + pair AllReduce ----------------
            st = small.tile([C, 2], f32, tag="st")
            nc.vector.tensor_reduce(st[:, 0:1], s1c[:], axis=mybir.AxisListType.X,
                                    op=Alu.add)
            nc.vector.tensor_reduce(st[:, 1:2], s2c[:], axis=mybir.AxisListType.X,
                                    op=Alu.add)
            gs_ps = op_pool.tile([GROUPS, 2], f32, tag="op")
            nc.tensor.matmul(gs_ps[:], gsels[:], st[:])
            gss = small.tile([GROUPS, 2], f32, tag="gss")
            nc.scalar.copy(gss[:], gs_ps[:])
            cci = drm.tile([GROUPS, 2], f32, tag="cci")
            cco = drm.tile([GROUPS, 2], f32, tag="cco")
            nc.sync.dma_start(cci[:], gss[:])
            nc.gpsimd.collective_compute(
                "AllReduce", Alu.add,
                replica_groups=[[0, 1], [2, 3], [4, 5], [6, 7]],
                ins=[cci.opt()], outs=[cco.opt()])
            gsr = small.tile([GROUPS, 2], f32, tag="gsr")
            nc.sync.dma_start(gsr[:], cco[:])
            ch_ps = op_pool.tile([C, 2], f32, tag="op")
            nc.tensor.matmul(ch_ps[:], gselTs[:], gsr[:])
            chs = small.tile([C, 2], f32, tag="chs")
            nc.scalar.activation(chs[:], ch_ps[:], Act.Copy, scale=1.0 / CNT)
            # scale = gamma * rsqrt(var + eps); shift = beta - mean*scale
            mean = chs[:, 0:1]
            tmp = small.tile([C, 1], f32, tag="tmp")
            nc.vector.tensor_tensor(out=tmp[:], in0=mean, in1=mean, op=Alu.mult)
            var = small.tile([C, 1], f32, tag="var")
            nc.vector.tensor_tensor(out=var[:], in0=chs[:, 1:2], in1=tmp[:],
                                    op=Alu.subtract)
            nc.vector.tensor_scalar(var[:], var[:], EPS, scalar2=None, op0=Alu.add)
            std = small.tile([C, 1], f32, tag="std")
            nc.scalar.activation(std[:], var[:], Act.Sqrt)
            rs = small.tile([C, 1], f32, tag="rs")
            nc.vector.reciprocal(rs[:], std[:])
            gnsc = small.tile([C, 1], f32, tag="gnsc")
            nc.vector.tensor_tensor(out=gnsc[:], in0=rs[:], in1=gnabs[:, 0:1],
                                    op=Alu.mult)
            gnsh = small.tile([C, 1], f32, tag="gnsh")
            nc.vector.tensor_tensor(out=gnsh[:], in0=mean, in1=gnsc[:], op=Alu.mult)
            nc.vector.tensor_tensor(out=gnsh[:], in0=gnabs[:, 1:2], in1=gnsh[:],
                                    op=Alu.subtract)

            # ---------------- phase 2 ----------------
            for i in range(NT):
                ys = ph2.tile([C, 1024], f32, tag="ys")
                nc.scalar.activation(ys[:], xtall[:, i * 1024:(i + 1) * 1024],
                                     Act.Prelu, bias=gnsh[:], scale=gnsc[:],
                                     alpha=float(prelu_alpha))
                mp = ph2.tile([C, 128], f32, tag="mp")
                nc.vector.tensor_reduce(mp[:], ys[:].rearrange("c (q s) -> c q s", s=NS),
                                        axis=mybir.AxisListType.X, op=Alu.max)
                o_ps = op_pool.tile([C, 128], f32, tag="op")
                nc.tensor.matmul(o_ps[:], wouts[:], mp[:])
                osb = ph2.tile([C, 128], f32, tag="osb")
                nc.vector.tensor_scalar(osb[:], o_ps[:], outbs[:], scalar2=None,
                                        op0=Alu.add)
                nc.sync.dma_start(d_out[:, i * 128:(i + 1) * 128], osb[:])

    nc.finalize()
    return nc


_CACHE = {}


def _bf16_split_pair(a5, b5):
    """3-level bf16 split of both operands of the g-matmul; all 9 level pairs
    stacked into K=45 so one bf16 matmul reproduces the fp32 product to
    ~2^-26 relative."""
    import ml_dtypes
    bf = ml_dtypes.bfloat16

    def split(v):
        h = v.astype(bf)
        m = (v - h.astype(np.float64)).astype(bf)
        l = (v - h.astype(np.float64) - m.astype(np.float64)).astype(bf)
        return h, m, l

    ah, am, al = split(a5)
    bh, bm, bl = split(b5)
    pairs = [(ah, bh), (ah, bm), (am, bh), (ah, bl), (al, bh),
             (am, bm), (am, bl), (al, bm), (al, bl)]
    a45 = np.concatenate([p[0] for p in pairs], axis=0)
    b45 = np.concatenate([p[1] for p in pairs], axis=0)
    return np.ascontiguousarray(a45), np.ascontiguousarray(b45)


def kernel(coords, xyz2, fmap1, fmap2, conv_w, conv_b, gn_gamma, gn_beta,
           prelu_a, out_w, out_b):
    from concourse.bass_utils import run_bass_kernel_spmd

    coords = np.asarray(coords, np.float32)
    xyz2 = np.asarray(xyz2, np.float32)
    fmap1 = np.asarray(fmap1, np.float32)
    fmap2 = np.asarray(fmap2, np.float32)
    conv_w = np.asarray(conv_w, np.float32)
    conv_b = np.asarray(conv_b, np.float32)
    gn_gamma = np.asarray(gn_gamma, np.float32)
    gn_beta = np.asarray(gn_beta, np.float32)
    prelu_a = np.asarray(prelu_a, np.float32)
    out_w = np.asarray(out_w, np.float32)
    out_b = np.asarray(out_b, np.float32)

    alpha = float(prelu_a.reshape(-1)[0])
    key = alpha
    if key not in _CACHE:
        _CACHE[key] = _build(alpha)
    nc = _CACHE[key]

    wcc = np.ascontiguousarray(conv_w.T[0:1, :], np.float32)   # [1, 64]
    wcx = np.ascontiguousarray(conv_w.T[1:4, :], np.float32)   # [3, 64]
    wcatb = np.concatenate([-conv_w[:, 1:4].T,
                            conv_b[None, :]], axis=0).astype(np.float32)  # [4, 64]
    wout = np.ascontiguousarray(out_w.T, np.float32)       # [64, 64]
    gnab = np.stack([gn_gamma, gn_beta], axis=1)           # [64, 2]
    outb = np.ascontiguousarray(out_b[:, None], np.float32)
    gsel = np.zeros((C, GROUPS), np.float32)
    gsel[np.arange(C), np.arange(C) // (C // GROUPS)] = 1.0
    gselT = np.ascontiguousarray(gsel.T)
    ones1 = np.ones((128, 1), np.float32)
    ident = np.eye(128, dtype=np.float32)

    in_maps = []
    for core in range(8):
        b, h = core // 2, core % 2
        qsl = slice(h * HALF, (h + 1) * HALF)
        cq3 = coords[b, qsl].astype(np.float64)            # [2048, 3]
        xyz = xyz2[b].astype(np.float64)                   # [4096, 3]
        gq5 = np.stack([2.0 * cq3[:, 0], 2.0 * cq3[:, 1], 2.0 * cq3[:, 2],
                        RADIUS * RADIUS - np.sum(cq3 * cq3, axis=1),
                        np.ones(HALF)], axis=0)
        cq = np.stack([cq3[:, 0], cq3[:, 1], cq3[:, 2], np.ones(HALF)],
                      axis=0).astype(np.float32)
        gp5 = np.stack([xyz[:, 0], xyz[:, 1], xyz[:, 2], np.ones(N),
                        -np.sum(xyz * xyz, axis=1)], axis=0)
        gq, gp = _bf16_split_pair(gq5, gp5)
        f1sc = np.ascontiguousarray(
            fmap1[b][:, qsl] * np.float32(1.0 / np.sqrt(DIM)), np.float32)
        xyzd = np.concatenate([np.ascontiguousarray(xyz2[b].T),
                               np.zeros((1, N), np.float32)], axis=0)
        in_maps.append({
            "gq": gq, "gp": gp, "cq": cq, "f1s": f1sc,
            "f2": np.ascontiguousarray(fmap2[b]), "xyzd": xyzd,
            "wcc": wcc, "wcx": wcx, "wcatb": wcatb, "wout": wout, "gnab": gnab, "outb": outb,
            "gsel": gsel, "gselT": gselT, "ones1": ones1, "ident": ident,
        })

    kres = run_bass_kernel_spmd(nc, in_maps, list(range(8)),
                                trace=bool(globals().get("TRACE", False)))
    globals()["LAST_RESULTS"] = kres
    res = kres.results

    out = np.empty((B, C, N), np.float32)
    for core in range(8):
        b, h = core // 2, core % 2
        out[b, :, h * HALF:(h + 1) * HALF] = res[core]["out"]
    return out
